# revision 1
# baseline (speedup 1.0000x reference)
"""Confusion-matrix metric kernel for Trainium2 (Bass/Tile), 8 NeuronCores.

Problem: prediction [N=262144, C=1000] f32, target [N] int -> CM [C, C] f32
where CM[t, p] = #{n : target_n == t and argmax(prediction_n) == p}.

Sharding (class-banded data-parallel): rows are bucketed by target band;
core k owns all rows with target in [125*k, 125*(k+1)).  Each core then
computes a DISJOINT 125-row slab of the confusion matrix, so the
all-reduce degenerates to stacking the 8 slabs (and its one-hot target
matmul only ever spans a 128-wide class band -> 8x less PE work).

Default algorithm (VARIANT=v3), two phases, both on-device:
  Phase 1 -- streaming, near the HBM roofline (~1.5us per [128,1000] tile):
    DVE: max8 (row max + runner-up) + is_ge mask for cols [0:split)
    ACT: |M-x| + relu(1-1e9*|M-x|) mask for cols [split:1000) and the
         one-hot(target) tile via |iota-t| + relu(1-a)
    PE : one-hot(t)^T @ mask accumulated in persistent PSUM banks
    The mask one-hot counts EVERY argmax position; rows where the row max
    occurs more than once (ties) are flagged by one batched runner-up==max
    compare and fixed in phase 2.  jnp.argmax keeps the first occurrence.
  Phase 2 -- one 128-row tile per core holding the flagged tie rows
    (~12 rows for the fixed seed): computes onehot(first-occurrence
    argmax via max8+max_index) minus the mask, scatters it through the
    same one-hot matmul, and the host adds the correction slab.
  Padding rows use local target class 126 (outside the 125-wide band),
  landing in a PSUM row that is never copied out.

VARIANT=v2 is a single-phase exact fallback (max8+max_index argmax per
tile, one-hot built on ACT); ~35% slower but no phase-2 machinery.
"""

import numpy as np

C = 1000
NCORES = 8
BAND = C // NCORES  # 125
P = 128
PAD_CLASS = 126  # local target class used for padding rows; never output

_BUILD_CACHE = {}


def _build(ntiles):
    """Build the Bass program for one core processing ntiles*128 rows."""
    from contextlib import ExitStack

    import concourse.bass as bass
    import concourse.tile as tile
    from concourse import bacc, mybir

    # Bacc (not raw Bass): its generate_event_semaphores pass splits
    # multi-sem waits, which TRN2 walrus codegen limits to 1 per instruction.
    nc = bacc.Bacc()
    rows = ntiles * P
    pred = nc.dram_tensor("pred", [rows, C], mybir.dt.float32, kind="ExternalInput")
    # tloc[p, i] = local target class of row i*128 + p (int32, in [0, 128))
    tloc = nc.dram_tensor("tloc", [P, ntiles], mybir.dt.int32, kind="ExternalInput")
    cm_out = nc.dram_tensor("cm", [BAND, C], mybir.dt.float32, kind="ExternalOutput")

    with ExitStack() as ctx:
        tc = ctx.enter_context(tile.TileContext(nc))
        const_pool = ctx.enter_context(tc.tile_pool(name="const", bufs=1))
        in_pool = ctx.enter_context(tc.tile_pool(name="inp", bufs=8))
        ohp_pool = ctx.enter_context(tc.tile_pool(name="ohp", bufs=4))
        ahp_pool = ctx.enter_context(tc.tile_pool(name="ahp", bufs=4))
        oht_pool = ctx.enter_context(tc.tile_pool(name="oht", bufs=4))
        small_pool = ctx.enter_context(tc.tile_pool(name="small", bufs=6))
        psum_pool = ctx.enter_context(
            tc.tile_pool(name="psum", bufs=1, space=bass.MemorySpace.PSUM)
        )

        # Constants: iota along the free dim, identical in every partition
        # (built as int32, cast once to f32 for is_equal compares).
        iota_c_i = const_pool.tile([P, C], mybir.dt.int32)
        nc.gpsimd.iota(iota_c_i[:], pattern=[[1, C]], base=0, channel_multiplier=0)
        iota_c = const_pool.tile([P, C], mybir.dt.float32)
        nc.vector.tensor_copy(iota_c[:], iota_c_i[:])
        iota_t_i = const_pool.tile([P, P], mybir.dt.int32)
        nc.gpsimd.iota(iota_t_i[:], pattern=[[1, P]], base=0, channel_multiplier=0)
        iota_t = const_pool.tile([P, P], mybir.dt.float32)
        nc.vector.tensor_copy(iota_t[:], iota_t_i[:])

        tloc_sb = const_pool.tile([P, ntiles], mybir.dt.int32)
        nc.sync.dma_start(tloc_sb[:], tloc[:])
        tloc_f = const_pool.tile([P, ntiles], mybir.dt.float32)
        nc.vector.tensor_copy(tloc_f[:], tloc_sb[:])

        # PSUM accumulator: columns 0:512 in bank 0, 512:1000 in bank 1.
        psum = psum_pool.tile([P, 1024], mybir.dt.float32)

        for i in range(ntiles):
            x = in_pool.tile([P, C], mybir.dt.float32)
            nc.gpsimd.dma_start(x[:], pred[i * P : (i + 1) * P, :])

            m8 = small_pool.tile([P, 8], mybir.dt.float32)
            nc.vector.max(m8[:], x[:])
            i8 = small_pool.tile([P, 8], mybir.dt.uint32)
            nc.vector.max_index(i8[:], m8[:], x[:])
            negidx = small_pool.tile([P, 1], mybir.dt.float32)
            nc.vector.tensor_scalar(
                negidx[:], i8[:, 0:1], -1.0, None, op0=mybir.AluOpType.mult
            )

            # one-hot(pred) on the otherwise-idle ACT engine:
            # a = |iota - idx| (integer-valued), ohp = relu(1 - a)
            a = ahp_pool.tile([P, C], mybir.dt.bfloat16)
            nc.scalar.activation(
                a[:], iota_c[:], mybir.ActivationFunctionType.Abs,
                bias=negidx[:, 0:1], scale=1.0,
            )
            ohp = ohp_pool.tile([P, C], mybir.dt.bfloat16)
            nc.scalar.activation(
                ohp[:], a[:], mybir.ActivationFunctionType.Relu,
                bias=1.0, scale=-1.0,
            )
            oht = oht_pool.tile([P, P], mybir.dt.bfloat16)
            nc.vector.tensor_scalar(
                oht[:], iota_t[:], tloc_f[:, i : i + 1], None,
                op0=mybir.AluOpType.is_equal,
            )

            first = i == 0
            last = i == ntiles - 1
            nc.tensor.matmul(
                psum[:, 0:512], oht[:], ohp[:, 0:512], start=first, stop=last
            )
            nc.tensor.matmul(
                psum[:, 512:1000], oht[:], ohp[:, 512:1000], start=first, stop=last
            )

        res = const_pool.tile([P, C], mybir.dt.float32)
        nc.scalar.copy(res[:], psum[:, 0:1000])
        nc.sync.dma_start(cm_out[:], res[0:BAND, :])

    nc.compile()
    return nc


def _build_v3(ntiles, split=660):
    """Phase-1 program: mask-based one-hot (ties over-counted, but counted
    via fused accumulators and fixed by the phase-2 program).

    Engine balance per [128, 1000] tile (~1.46 us DMA roofline):
      DVE: reduce_max (~1.1us) + is_ge mask cols 0:split + oht
      ACT: Abs + Relu mask cols split:1000 (~1.2ns/col total)
      PE : one-hot matmuls into the persistent PSUM accumulator
    `split` balances DVE vs ACT; psum bank boundary stays at column 512.
    """
    from contextlib import ExitStack

    import concourse.bass as bass
    import concourse.tile as tile
    from concourse import bacc, mybir

    nc = bacc.Bacc()
    rows = ntiles * P
    pred = nc.dram_tensor("pred", [rows, C], mybir.dt.float32, kind="ExternalInput")
    tloc = nc.dram_tensor("tloc", [P, ntiles], mybir.dt.int32, kind="ExternalInput")
    cm_out = nc.dram_tensor("cm", [BAND, C], mybir.dt.float32, kind="ExternalOutput")
    tie_out = nc.dram_tensor(
        "tie", [P, ntiles], mybir.dt.float32, kind="ExternalOutput"
    )

    with ExitStack() as ctx:
        tc = ctx.enter_context(tile.TileContext(nc))
        const_pool = ctx.enter_context(tc.tile_pool(name="const", bufs=1))
        in_pool = ctx.enter_context(tc.tile_pool(name="inp", bufs=12))
        lo_pool = ctx.enter_context(tc.tile_pool(name="lo", bufs=4))
        hi_pool = ctx.enter_context(tc.tile_pool(name="hi", bufs=4))
        ahi_pool = ctx.enter_context(tc.tile_pool(name="ahi", bufs=4))
        oht_pool = ctx.enter_context(tc.tile_pool(name="oht", bufs=4))
        small_pool = ctx.enter_context(tc.tile_pool(name="small", bufs=6))
        psum_pool = ctx.enter_context(
            tc.tile_pool(name="psum", bufs=1, space=bass.MemorySpace.PSUM)
        )

        iota_t_i = const_pool.tile([P, P], mybir.dt.int32)
        nc.gpsimd.iota(iota_t_i[:], pattern=[[1, P]], base=0, channel_multiplier=0)
        iota_t = const_pool.tile([P, P], mybir.dt.float32)
        nc.vector.tensor_copy(iota_t[:], iota_t_i[:])

        tloc_sb = const_pool.tile([P, ntiles], mybir.dt.int32)
        nc.sync.dma_start(tloc_sb[:], tloc[:])
        # negated local targets, f32: ACT builds oht = relu(1 - |iota - t|)
        tloc_n = const_pool.tile([P, ntiles], mybir.dt.float32)
        nc.vector.tensor_scalar(
            tloc_n[:], tloc_sb[:], -1.0, None, op0=mybir.AluOpType.mult
        )

        tie_all = const_pool.tile([P, ntiles], mybir.dt.float32)
        # all max8 outputs live here; [:, i, 0] is tile i's row max and
        # [:, i, 1] the runner-up (equal iff tie) -- compared once at the end
        m8_all = const_pool.tile([P, ntiles, 8], mybir.dt.float32)

        # 3 PSUM banks: [0:512]=CM[0:512], [512:1024]=CM[split:1000],
        # [1024:1536]=CM[512:split] -- one accumulation group per bank.
        psum = psum_pool.tile([P, 1536], mybir.dt.float32)

        for i in range(ntiles):
            x = in_pool.tile([P, C], mybir.dt.float32)
            nc.gpsimd.dma_start(x[:], pred[i * P : (i + 1) * P, :])

            # top-8 row values
            nc.vector.max(m8_all[:, i], x[:])
            m_i = m8_all[:, i, 0:1]

            # low cols on DVE: mask = (x >= M)  (2x mode, no accumulator)
            ohp_lo = lo_pool.tile([P, split], mybir.dt.bfloat16)
            nc.vector.tensor_scalar(
                ohp_lo[:], x[:, 0:split], m_i, None,
                op0=mybir.AluOpType.is_ge,
            )
            # high cols on ACT: a = |M - x| (0 iff max), mask = relu(1 - 1e9*a)
            a_hi = ahi_pool.tile([P, C - split], mybir.dt.bfloat16)
            nc.scalar.activation(
                a_hi[:], x[:, split:C], mybir.ActivationFunctionType.Abs,
                bias=m_i, scale=-1.0,
            )
            ohp_hi = hi_pool.tile([P, C - split], mybir.dt.bfloat16)
            nc.scalar.activation(
                ohp_hi[:], a_hi[:], mybir.ActivationFunctionType.Relu,
                bias=1.0, scale=-1e9,
            )

            # oht on ACT too: a_t = |iota_t - t|, oht = relu(1 - a_t)
            a_t = oht_pool.tile([P, P], mybir.dt.bfloat16, tag="a_t")
            nc.scalar.activation(
                a_t[:], iota_t[:], mybir.ActivationFunctionType.Abs,
                bias=tloc_n[:, i : i + 1], scale=1.0,
            )
            oht = oht_pool.tile([P, P], mybir.dt.bfloat16, tag="oht")
            nc.scalar.activation(
                oht[:], a_t[:], mybir.ActivationFunctionType.Relu,
                bias=1.0, scale=-1.0,
            )

            first = i == 0
            last = i == ntiles - 1
            assert split >= 512
            nc.tensor.matmul(
                psum[:, 0:512], oht[:], ohp_lo[:, 0:512], start=first, stop=last
            )
            if split > 512:
                nc.tensor.matmul(
                    psum[:, 1024 : 1024 + (split - 512)], oht[:],
                    ohp_lo[:, 512:split], start=first, stop=last,
                )
            nc.tensor.matmul(
                psum[:, 512 : 512 + (C - split)], oht[:], ohp_hi[:],
                start=first, stop=last,
            )

        # one batched tie test for all tiles: runner-up == max per row
        nc.vector.tensor_tensor(
            tie_all[:], m8_all[:, :, 1], m8_all[:, :, 0],
            op=mybir.AluOpType.is_ge,
        )

        res = const_pool.tile([P, C], mybir.dt.float32)
        nc.scalar.copy(res[:, 0:512], psum[:, 0:512])
        if split > 512:
            nc.scalar.copy(res[:, 512:split], psum[:, 1024 : 1024 + (split - 512)])
        nc.scalar.copy(res[:, split:C], psum[:, 512 : 512 + (C - split)])
        nc.sync.dma_start(cm_out[:], res[0:BAND, :])
        nc.sync.dma_start(tie_out[:], tie_all[:])

    nc.compile()
    return nc


def _build_fix():
    """Phase-2 program: one 128-row tile of tie rows.  Computes
    correction = onehot(first-occurrence argmax) - mask, scattered through
    the same one-hot matmul; host adds the correction slab to phase 1's."""
    from contextlib import ExitStack

    import concourse.bass as bass
    import concourse.tile as tile
    from concourse import bacc, mybir

    nc = bacc.Bacc()
    pred = nc.dram_tensor("pred", [P, C], mybir.dt.float32, kind="ExternalInput")
    tloc = nc.dram_tensor("tloc", [P, 1], mybir.dt.int32, kind="ExternalInput")
    corr_out = nc.dram_tensor(
        "corr", [BAND, C], mybir.dt.float32, kind="ExternalOutput"
    )

    with ExitStack() as ctx:
        tc = ctx.enter_context(tile.TileContext(nc))
        pool = ctx.enter_context(tc.tile_pool(name="pool", bufs=1))
        psum_pool = ctx.enter_context(
            tc.tile_pool(name="psum", bufs=1, space=bass.MemorySpace.PSUM)
        )

        iota_c_i = pool.tile([P, C], mybir.dt.int32)
        nc.gpsimd.iota(iota_c_i[:], pattern=[[1, C]], base=0, channel_multiplier=0)
        iota_c = pool.tile([P, C], mybir.dt.float32)
        nc.vector.tensor_copy(iota_c[:], iota_c_i[:])
        iota_t_i = pool.tile([P, P], mybir.dt.int32)
        nc.gpsimd.iota(iota_t_i[:], pattern=[[1, P]], base=0, channel_multiplier=0)
        iota_t = pool.tile([P, P], mybir.dt.float32)
        nc.vector.tensor_copy(iota_t[:], iota_t_i[:])

        tloc_sb = pool.tile([P, 1], mybir.dt.int32)
        nc.sync.dma_start(tloc_sb[:], tloc[:])
        tloc_f = pool.tile([P, 1], mybir.dt.float32)
        nc.vector.tensor_copy(tloc_f[:], tloc_sb[:])

        x = pool.tile([P, C], mybir.dt.float32)
        nc.gpsimd.dma_start(x[:], pred[:])

        m = pool.tile([P, 1], mybir.dt.float32)
        nc.vector.reduce_max(m[:], x[:], axis=mybir.AxisListType.X)
        mask = pool.tile([P, C], mybir.dt.bfloat16)
        nc.vector.tensor_scalar(
            mask[:], x[:], m[:, 0:1], None, op0=mybir.AluOpType.is_ge
        )

        m8 = pool.tile([P, 8], mybir.dt.float32)
        nc.vector.max(m8[:], x[:])
        i8 = pool.tile([P, 8], mybir.dt.uint32)
        nc.vector.max_index(i8[:], m8[:], x[:])
        idxf = pool.tile([P, 1], mybir.dt.float32)
        nc.vector.tensor_copy(idxf[:], i8[:, 0:1])
        ohp = pool.tile([P, C], mybir.dt.bfloat16)
        nc.vector.tensor_scalar(
            ohp[:], iota_c[:], idxf[:, 0:1], None, op0=mybir.AluOpType.is_equal
        )

        diff = pool.tile([P, C], mybir.dt.bfloat16)
        nc.vector.tensor_sub(diff[:], ohp[:], mask[:])

        oht = pool.tile([P, P], mybir.dt.bfloat16)
        nc.vector.tensor_scalar(
            oht[:], iota_t[:], tloc_f[:, 0:1], None, op0=mybir.AluOpType.is_equal
        )

        psum = psum_pool.tile([P, 1024], mybir.dt.float32)
        nc.tensor.matmul(psum[:, 0:512], oht[:], diff[:, 0:512], start=True, stop=True)
        nc.tensor.matmul(
            psum[:, 512:1000], oht[:], diff[:, 512:1000], start=True, stop=True
        )

        res = pool.tile([P, C], mybir.dt.float32)
        nc.scalar.copy(res[:], psum[:, 0:1000])
        nc.sync.dma_start(corr_out[:], res[0:BAND, :])

    nc.compile()
    return nc


def _get_program(ntiles):
    if ntiles not in _BUILD_CACHE:
        _BUILD_CACHE[ntiles] = _build(ntiles)
    return _BUILD_CACHE[ntiles]


def _get_program_v3(ntiles):
    key = ("v3", ntiles)
    if key not in _BUILD_CACHE:
        _BUILD_CACHE[key] = _build_v3(ntiles)
    return _BUILD_CACHE[key]


def _get_fix_program():
    if "fix" not in _BUILD_CACHE:
        _BUILD_CACHE["fix"] = _build_fix()
    return _BUILD_CACHE["fix"]


def _shard_inputs(prediction, target):
    """Bucket rows by target band; pad each core to a common tile count."""
    target = np.asarray(target).astype(np.int64).reshape(-1)
    prediction = np.asarray(prediction, dtype=np.float32)
    n = prediction.shape[0]
    assert target.shape[0] == n and prediction.shape[1] == C

    band = target // BAND
    idxs = [np.nonzero(band == k)[0] for k in range(NCORES)]
    maxcnt = max(len(ix) for ix in idxs)
    ntiles = max(1, -(-maxcnt // P))
    rows = ntiles * P

    in_maps = []
    for k in range(NCORES):
        ix = idxs[k]
        pk = np.zeros((rows, C), np.float32)
        if len(ix):
            np.take(prediction, ix, axis=0, out=pk[: len(ix)])
        tk = np.full((rows,), PAD_CLASS, np.int32)
        tk[: len(ix)] = (target[ix] - k * BAND).astype(np.int32)
        tl = np.ascontiguousarray(tk.reshape(ntiles, P).T)
        in_maps.append({"pred": pk, "tloc": tl})
    lens = [len(ix) for ix in idxs]
    return in_maps, ntiles, lens


import os as _os

VARIANT = _os.environ.get("CM_KERNEL_VARIANT", "v3")


def kernel(prediction, target, num_classes=C, _trace=False, _tmpdir=None):
    num_classes = int(num_classes)
    assert num_classes == C, f"kernel hardcoded for C={C}, got {num_classes}"

    in_maps, ntiles, lens = _shard_inputs(prediction, target)

    from concourse.bass_utils import run_bass_kernel_spmd

    cores = list(range(NCORES))
    kw = {}
    if _trace:
        kw = dict(trace=True, trace_cores=cores, tmpdir=_tmpdir)
    reslist = []

    if VARIANT == "v2":
        nc = _get_program(ntiles)
        res = run_bass_kernel_spmd(nc, in_maps, core_ids=cores, **kw)
        reslist.append(res)
        cm = np.concatenate([res.results[k]["cm"] for k in range(NCORES)], axis=0)
    else:
        nc = _get_program_v3(ntiles)
        res = run_bass_kernel_spmd(nc, in_maps, core_ids=cores, **kw)
        reslist.append(res)
        cm = np.concatenate([res.results[k]["cm"] for k in range(NCORES)], axis=0)

        # tie rows: positions where the row max occurs more than once.
        # (mask-based one-hot counted every occurrence; fix via phase 2)
        tie_rows = []
        any_tie = False
        for k in range(NCORES):
            tie = res.results[k]["tie"].T.reshape(-1)[: lens[k]]
            rows_k = np.nonzero(tie > 0.5)[0]
            tie_rows.append(rows_k)
            any_tie = any_tie or len(rows_k) > 0

        if any_tie:
            ncf = _get_fix_program()
            fix_maps = []
            for k in range(NCORES):
                rows_k = tie_rows[k][:P]
                pk = np.zeros((P, C), np.float32)
                tk = np.full((P, 1), PAD_CLASS, np.int32)
                if len(rows_k):
                    pk[: len(rows_k)] = in_maps[k]["pred"][rows_k]
                    tl = in_maps[k]["tloc"]  # [P, ntiles]
                    tk[: len(rows_k), 0] = tl[rows_k % P, rows_k // P]
                fix_maps.append({"pred": pk, "tloc": tk})
            kwf = {}
            if _trace:
                fixdir = (_tmpdir + "_fix") if _tmpdir else None
                if fixdir:
                    import shutil

                    shutil.rmtree(fixdir, ignore_errors=True)
                    _os.makedirs(fixdir, exist_ok=True)
                kwf = dict(trace=True, trace_cores=cores, tmpdir=fixdir)
            res2 = run_bass_kernel_spmd(ncf, fix_maps, core_ids=cores, **kwf)
            reslist.append(res2)
            for k in range(NCORES):
                cm[k * BAND : (k + 1) * BAND] += res2.results[k]["corr"]

    out = np.ascontiguousarray(cm, dtype=np.float32)
    if _trace:
        return out, reslist
    return out

